# revision 1
# baseline (speedup 1.0000x reference)
"""AttnBlock (GroupNorm -> single-head 4096x4096 attention -> proj -> residual)
on x:[2,512,64,64] f32, distributed over 8 trn2 NeuronCores.

Sharding: data-parallel over batch (2) x sequence-parallel over query rows
(4 chunks of 1024). Each core receives its batch's full [512, 4096] image with
spatial columns permuted so that its own 1024 query positions are columns
0:1024 (attention and groupnorm are permutation-invariant over spatial
positions, which keeps the SPMD program identical across cores).

x is DMA'd once and stays resident in SBUF. GroupNorm is folded into the
q/k/v weights on device: h = (x-mu)*rstd, so W^T rows are scaled by rstd
(per input channel = per partition) and the biases pick up a -W'^T mu
correction computed with one thin matmul per weight. The conv/attention
matmuls then consume raw x directly.

Matmuls run as float32r (full-rate fp32 path on the PE); the BIR verifier
requires every f32r matmul operand to be produced by a rounding instruction
(DVE/ACT write or a DMA whose source is already f32r), which drives the
tile dtypes below.
"""

import numpy as np

import concourse.bass as bass
import concourse.mybir as mybir
import concourse.tile as tile
from concourse import bacc
from concourse.bass_utils import run_bass_kernel_spmd

F32 = mybir.dt.float32
F32R = mybir.dt.float32r
BF16 = mybir.dt.bfloat16

# When True, all matmul-facing tiles are bf16: FWL-accelerated weight loads,
# V^T kept SBUF-resident (no DRAM spill), half the K/Q footprint. PSUM stays
# fp32. Toggled after A/B measurement of speed vs accuracy.
ATT_BF16 = False

# MIXED: K and V^T tiles bf16, rest f32r. REJECTED: walrus forbids mixing
# 32-bit and non-32-bit matmul inputs (NCC_IBIR034).
MIXED = False

# ABF16: attention-only bf16 — K/Q/V^T/P tiles bf16 (V^T SBUF-resident, no
# DRAM spill), convs and proj stay f32r so K/Q/V^T content is computed at
# full precision and only storage-quantized.
ABF16 = False

B = 2
C = 512
H = 64
W = 64
N = H * W            # 4096 spatial positions
G = 32               # groups
EPS = 1e-6
CH = 4               # channel chunks of 128
NS = 8               # j slices of 512
JT = 32              # j tiles of 128
I = 1024             # query positions per core
IC = 2               # i chunks of 512 per core
SCALE = float(C) ** -0.5

_cached = {}


def _build(repeat=1):
    MDT = BF16 if ATT_BF16 else F32R
    KDT = BF16 if (ATT_BF16 or MIXED or ABF16) else F32R
    QDT = BF16 if (ATT_BF16 or ABF16) else F32R
    vt_res = ATT_BF16 or MIXED or ABF16
    nc = bacc.Bacc("TRN2", target_bir_lowering=False, debug=False, num_devices=8)

    x_d = nc.dram_tensor("x", [C, N], F32R, kind="ExternalInput").ap()
    wq_d = nc.dram_tensor("wqt", [C, C], MDT, kind="ExternalInput").ap()
    wk_d = nc.dram_tensor("wkt", [C, C], MDT, kind="ExternalInput").ap()
    wv_d = nc.dram_tensor("wvt", [C, C], MDT, kind="ExternalInput").ap()
    wp_d = nc.dram_tensor("wpt", [C, C], MDT, kind="ExternalInput").ap()
    bq_d = nc.dram_tensor("bq", [C], F32, kind="ExternalInput").ap()
    bk_d = nc.dram_tensor("bk", [C], F32, kind="ExternalInput").ap()
    bv_d = nc.dram_tensor("bv", [C], F32, kind="ExternalInput").ap()
    bp_d = nc.dram_tensor("bp", [C], F32, kind="ExternalInput").ap()
    gm_d = nc.dram_tensor("gmat", [128, 128], F32, kind="ExternalInput").ap()
    out_ds = [
        nc.dram_tensor("out" if r == 0 else f"out{r}", [C, I], F32,
                       kind="ExternalOutput").ap()
        for r in range(repeat)
    ]

    x_r = x_d.rearrange("(ch p) n -> p ch n", p=128)       # [128, 4, 4096]

    from contextlib import ExitStack
    with tile.TileContext(nc) as tc:
      for rep in range(repeat):
        sfx = f"_{rep}"
        out_r = out_ds[rep].rearrange("(ch p) i -> p ch i", p=128)
        ctx_psum = ExitStack()
        with (
            tc.tile_pool(name="consts" + sfx + sfx, bufs=1) as consts,
            tc.tile_pool(name="big" + sfx + sfx, bufs=1) as big,
            tc.tile_pool(name="stats" + sfx + sfx, bufs=1) as stats,
            tc.tile_pool(name="small" + sfx + sfx, bufs=1) as small,
            tc.tile_pool(name="fin" + sfx + sfx, bufs=2) as fin,
            tc.tile_pool(name="dram" + sfx + sfx, bufs=1, space="DRAM") as dram,
        ):
            # ---- persistent constants ----
            bp_sb = consts.tile([128, CH], F32, tag="bp")
            nc.sync.dma_start(out=bp_sb[:], in_=bp_d.rearrange("(ch p) -> p ch", p=128))
            gm_sb = consts.tile([128, 128], F32, tag="gm")
            nc.sync.dma_start(out=gm_sb[:], in_=gm_d)
            eps_sb = consts.tile([128, 1], F32, tag="eps")
            nc.vector.memset(eps_sb[:], EPS)
            ones_c = consts.tile([128, 1], F32, tag="onesc")
            nc.vector.memset(ones_c[:], 1.0)
            ones_r = consts.tile([1, 128], F32, tag="onesr")
            nc.vector.memset(ones_r[:], 1.0)

            X_tiles = [
                big.tile([128, CH, 512], F32R, tag=f"X{s}", name=f"X{s}" + sfx)
                for s in range(NS)
            ]  # raw x, resident, one tile per j-slice for fine-grained deps
            K_sb = big.tile([128, CH, N], KDT, tag="K")        # K[c, j]
            Q_sb = big.tile([128, CH, I], QDT, tag="Q")        # Q[c, i]
            if vt_res:
                VT_sb = big.tile([128, JT, C], BF16, tag="VT")  # V^T resident
            else:
                vt_spill = dram.tile([JT, 128, C], F32R, tag="vts")  # V^T[j, c]
            bck_scratch = dram.tile([1, C], F32, tag="bcsk")  # bias-corr transpose (k)
            bcq_scratch = dram.tile([1, C], F32, tag="bcsq")  # bias-corr transpose (q)

            # groupnorm stats tiles
            st_acc = stats.tile([128, CH, NS, 6], F32, tag="stacc")
            mv = stats.tile([128, CH, 2], F32, tag="mv")
            gs = stats.tile([128, CH, 2], F32, tag="gs")
            mean_sb = stats.tile([128, CH], F32, tag="mean")
            rstd_sb = stats.tile([128, CH], F32, tag="rstd")

            psc = ctx_psum.enter_context(
                tc.tile_pool(name="psc" + sfx + sfx, bufs=4, space="PSUM")
            )
            with (
                tc.tile_pool(name="wtmp" + sfx + sfx, bufs=1) as wtmp,
                tc.tile_pool(name="vtev" + sfx + sfx, bufs=3) as vtev,
            ):
                # ---- phase 1: groupnorm stats over resident x ----
                for s in range(NS):
                    dma_eng = nc.sync if s % 2 == 0 else nc.gpsimd
                    dma_eng.dma_start(
                        out=X_tiles[s][:], in_=x_r[:, :, s * 512:(s + 1) * 512],
                    )
                    for ch in range(CH):
                        nc.vector.bn_stats(
                            out=st_acc[:, ch, s, :],
                            in_=X_tiles[s][:, ch, :].bitcast(F32),
                        )
                # weights are needed only after the stats chain; emitting their
                # DMAs here keeps the x stream at the head of the DMA queue
                wq_sb = [wtmp.tile([128, C], MDT, tag=f"wq{c}", name=f"wq{c}" + sfx) for c in range(CH)]
                wk_sb = [wtmp.tile([128, C], MDT, tag=f"wk{c}", name=f"wk{c}" + sfx) for c in range(CH)]
                wv_sb = [wtmp.tile([128, C], MDT, tag=f"wv{c}", name=f"wv{c}" + sfx) for c in range(CH)]
                for w_sb, w_d in ((wk_sb, wk_d), (wv_sb, wv_d), (wq_sb, wq_d)):
                    w_r = w_d.rearrange("(ch p) o -> p ch o", p=128)
                    for ch in range(CH):
                        nc.sync.dma_start(out=w_sb[ch][:], in_=w_r[:, ch, :])
                bq_row = wtmp.tile([1, C], F32, tag="bqr")
                bk_row = wtmp.tile([1, C], F32, tag="bkr")
                bv_row = wtmp.tile([1, C], F32, tag="bvr")
                for b_sb, b_d in ((bq_row, bq_d), (bk_row, bk_d), (bv_row, bv_d)):
                    nc.sync.dma_start(out=b_sb[:], in_=b_d[None, :])

                for ch in range(CH):
                    nc.vector.bn_aggr(out=mv[:, ch, :], in_=st_acc[:, ch, :, :])
                # per-channel (mean, E[x^2]) -> group-averaged via gmat matmul
                nc.vector.tensor_copy(out=gs[:, :, 0], in_=mv[:, :, 0])
                nc.vector.tensor_mul(out=gs[:, :, 1], in0=mv[:, :, 0], in1=mv[:, :, 0])
                nc.vector.tensor_add(out=gs[:, :, 1], in0=gs[:, :, 1], in1=mv[:, :, 1])
                pg = psc.tile([128, CH, 2], F32, tag="pc", bufs=4)
                nc.tensor.matmul(pg[:], gm_sb[:], gs[:], start=True, stop=True)
                nc.vector.tensor_copy(out=mean_sb[:], in_=pg[:, :, 0])
                nc.vector.tensor_mul(out=rstd_sb[:], in0=mean_sb[:], in1=mean_sb[:])
                nc.vector.tensor_sub(out=rstd_sb[:], in0=pg[:, :, 1], in1=rstd_sb[:])
                nc.scalar.activation(
                    out=rstd_sb[:], in_=rstd_sb[:],
                    func=mybir.ActivationFunctionType.Sqrt, bias=eps_sb[:],
                )
                nc.vector.reciprocal(out=rstd_sb[:], in_=rstd_sb[:])

                # ---- fold groupnorm into weights: w' = w * rstd(c_in) ----
                for w_sb in (wk_sb, wv_sb, wq_sb):
                    for ch in range(CH):
                        nc.vector.tensor_scalar_mul(
                            out=w_sb[ch][:], in0=w_sb[ch][:],
                            scalar1=rstd_sb[:, ch:ch + 1],
                        )
                # bias corrections d[o] = sum_c w'[c,o] * mu(c), as [1, 512]
                mean_r = stats.tile([128, CH], MDT, tag="meanr")
                nc.vector.tensor_copy(out=mean_r[:], in_=mean_sb[:])
                dcorr = {}
                for nm, w_sb in (("k", wk_sb), ("v", wv_sb), ("q", wq_sb)):
                    pd = psc.tile([128, 512], F32, tag="pd", name=f"pd_{nm}" + sfx, bufs=2)
                    for ch in range(CH):
                        nc.tensor.matmul(
                            pd[:1, :], mean_r[:, ch:ch + 1], w_sb[ch][:],
                            start=(ch == 0), stop=(ch == CH - 1),
                        )
                    dcorr[nm] = pd
                # corrected row biases: b' = b - d (in place), then transpose
                # via DRAM roundtrip to per-partition layout; k is independent
                # of q so the K-conv evicts don't wait on the q weight DMA
                nc.vector.tensor_sub(out=bk_row[:], in0=bk_row[:], in1=dcorr["k"][:1, :])
                nc.sync.dma_start(out=bck_scratch[:], in_=bk_row[:])
                bcorr_k = small.tile([128, CH], F32, tag="bcorrk")
                nc.sync.dma_start(
                    out=bcorr_k[:],
                    in_=bass.AP(
                        tensor=bck_scratch.tensor, offset=bck_scratch.offset,
                        ap=[[1, 128], [128, CH]],
                    ),
                )
                nc.vector.tensor_sub(out=bv_row[:], in0=bv_row[:], in1=dcorr["v"][:1, :])
                nc.vector.tensor_sub(out=bq_row[:], in0=bq_row[:], in1=dcorr["q"][:1, :])
                nc.sync.dma_start(out=bcq_scratch[:], in_=bq_row[:])
                bcorr_q = small.tile([128, CH], F32, tag="bcorrq")
                nc.sync.dma_start(
                    out=bcorr_q[:],
                    in_=bass.AP(
                        tensor=bcq_scratch.tensor, offset=bcq_scratch.offset,
                        ap=[[1, 128], [128, CH]],
                    ),
                )
                # v bias is broadcast along partitions (free dim = c_out)
                pbv = psc.tile([128, 512], F32, tag="pd", name="pd_bv" + sfx, bufs=2)
                nc.tensor.matmul(pbv[:], ones_r[:], bv_row[:], start=True, stop=True)
                bvv_bc = small.tile([128, C], F32, tag="bvvbc")
                nc.vector.tensor_copy(out=bvv_bc[:], in_=pbv[:])

                # ---- phase 2: K / V^T / Q convs from raw x + folded weights ----
                def x_for_mm(s):
                    if not ATT_BF16:
                        return X_tiles[s]
                    xb = vtev.tile([128, CH, 512], BF16, tag="xb",
                                   name=f"xb_{s}_{np_rand_tag[0]}" + sfx)
                    np_rand_tag[0] += 1
                    nc.vector.tensor_copy(out=xb[:], in_=X_tiles[s][:].bitcast(F32))
                    return xb

                np_rand_tag = [0]
                for s in range(NS):
                    xsl = x_for_mm(s)
                    # K[c_out, j_slice]
                    for t in range(CH):
                        pk = psc.tile([128, 512], F32, tag="pc")
                        for ch in range(CH):
                            nc.tensor.matmul(
                                pk[:], wk_sb[ch][:, t * 128:(t + 1) * 128],
                                xsl[:, ch, :], start=(ch == 0), stop=(ch == CH - 1),
                            )
                        nc.vector.tensor_scalar_add(
                            out=K_sb[:, t, s * 512:(s + 1) * 512], in0=pk[:],
                            scalar1=bcorr_k[:, t:t + 1],
                        )
                    # V^T[j_tile, c] -> spill to DRAM
                    for jj in range(4):
                        jt = 4 * s + jj
                        pv = psc.tile([128, 512], F32, tag="pc")
                        for ch in range(CH):
                            nc.tensor.matmul(
                                pv[:], xsl[:, ch, jj * 128:(jj + 1) * 128],
                                wv_sb[ch][:], start=(ch == 0), stop=(ch == CH - 1),
                            )
                        if vt_res:
                            nc.vector.tensor_add(
                                out=VT_sb[:, jt, :], in0=pv[:], in1=bvv_bc[:]
                            )
                        else:
                            vt = vtev.tile([128, C], F32R, tag="vt")
                            nc.vector.tensor_add(out=vt[:], in0=pv[:], in1=bvv_bc[:])
                            nc.sync.dma_start(out=vt_spill[jt], in_=vt[:])
                    # Q convs ride mid-sweep, once bcorr_q has surely landed
                    if s == 3:
                        xq_mm = {sq: x_for_mm(sq) for sq in range(IC)}
                        for sq in range(IC):
                            for t in range(CH):
                                pq = psc.tile([128, 512], F32, tag="pd", name=f"pq_{sq}_{t}" + sfx, bufs=2)
                                for ch in range(CH):
                                    nc.tensor.matmul(
                                        pq[:], wq_sb[ch][:, t * 128:(t + 1) * 128],
                                        xq_mm[sq][:, ch, :], start=(ch == 0), stop=(ch == CH - 1),
                                    )
                                nc.vector.tensor_scalar_add(
                                    out=Q_sb[:, t, sq * 512:(sq + 1) * 512], in0=pq[:],
                                    scalar1=bcorr_q[:, t:t + 1],
                                )


            # wp is needed only at proj time; its DMA rides under the conv phase
            wp_sb = consts.tile([128, CH, C], MDT, tag="wp")
            nc.sync.dma_start(out=wp_sb[:], in_=wp_d.rearrange("(ch p) o -> p ch o", p=128))

            # ---- phase 3: attention + proj, per i-chunk of 512 ----
            # (reuses the unified PSUM pool: po->pc slots, ps->pd, pm->pg)
            with (
                tc.tile_pool(name="vtst" + sfx + sfx, bufs=3) as vtst,
                tc.tile_pool(name="pexp" + sfx + sfx, bufs=2) as pexp,
                tc.tile_pool(name="osb" + sfx + sfx, bufs=4) as osb,
            ):
                ps_tiles = {}
                emitted = set()
                NPAIR = JT // 2

                def emit_s(ic, pr):
                    # one S-pair: two j-tiles into a double-wide (2-bank) psum
                    emitted.add((ic, pr))
                    qs2 = Q_sb[:, :, ic * 512:(ic + 1) * 512]
                    ps = psc.tile([128, 2, 512], F32, tag="pd", name=f"ps_{ic}_{pr}" + sfx, bufs=2)
                    for u in range(2):
                        jt = 2 * pr + u
                        for ch in range(CH):
                            nc.tensor.matmul(
                                ps[:, u, :], K_sb[:, ch, jt * 128:(jt + 1) * 128],
                                qs2[:, ch, :], start=(ch == 0), stop=(ch == CH - 1),
                            )
                    ps_tiles[(ic, pr)] = ps

                emit_s(0, 0)
                for ic in range(IC):
                    qs = Q_sb[:, :, ic * 512:(ic + 1) * 512]
                    po = [
                        psc.tile([128, 512], F32, tag="pc", name=f"po_{ic}_{ct}" + sfx)
                        for ct in range(CH)
                    ]
                    rs_parts = [
                        small.tile([128, 512], F32, tag=f"rsacc{k}", name=f"rs_{ic}_{k}" + sfx)
                        for k in range(2)
                    ]

                    # software-pipelined: emit S(jt+1) before O(jt) so the PE
                    # never waits on the ACT exp of the current tile; at the
                    # end of a chunk, prefetch the next chunk's first S tiles
                    # so the PE has work during the DVE-heavy epilogue
                    for pr in range(NPAIR):
                        # one exp instruction covers both j-tiles of the pair
                        pt = pexp.tile([128, 2, 512], QDT, tag="pt", name=f"pt_{ic}_{pr}" + sfx)
                        nc.scalar.activation(
                            out=pt[:], in_=ps_tiles.pop((ic, pr))[:],
                            func=mybir.ActivationFunctionType.Exp, scale=SCALE,
                        )
                        if pr + 1 < NPAIR:
                            if (ic, pr + 1) not in emitted:
                                emit_s(ic, pr + 1)
                        elif ic + 1 < IC:
                            emit_s(ic + 1, 0)
                        if vt_res:
                            vtp = None
                        else:
                            vtp = vtst.tile([128, 2, C], F32R, tag="vst", name=f"vst_{ic}_{pr}" + sfx)
                            nc.sync.dma_start(
                                out=vtp[:],
                                in_=vt_spill[2 * pr:2 * pr + 2].rearrange("two p c -> p two c"),
                            )
                        for u in range(2):
                            jt = 2 * pr + u
                            vt = VT_sb[:, jt, :] if vt_res else vtp[:, u, :]
                            for ct in range(CH):
                                nc.tensor.matmul(
                                    po[ct][:], vt[:, ct * 128:(ct + 1) * 128], pt[:, u, :],
                                    start=(jt == 0), stop=(jt == JT - 1),
                                )
                            ph = pt[:, u, :] if (ATT_BF16 or ABF16) else pt[:, u, :].bitcast(F32)
                            rs_k = rs_parts[u]
                            if pr == 0:
                                nc.vector.tensor_copy(out=rs_k[:], in_=ph)
                            else:
                                nc.vector.tensor_add(out=rs_k[:], in0=rs_k[:], in1=ph)

                    # row sums -> reciprocal -> broadcast to all partitions
                    nc.vector.tensor_add(
                        out=rs_parts[0][:], in0=rs_parts[0][:], in1=rs_parts[1][:]
                    )
                    prs = psc.tile([128, 512], F32, tag="pc", name=f"prs_{ic}" + sfx)
                    nc.tensor.matmul(prs[:1, :], ones_c[:], rs_parts[0][:], start=True, stop=True)
                    rinv = small.tile([1, 512], F32, tag="rinv")
                    nc.vector.reciprocal(out=rinv[:], in_=prs[:1, :])
                    pbc = psc.tile([128, 512], F32, tag="pc", name=f"pbc_{ic}" + sfx)
                    nc.tensor.matmul(pbc[:], ones_r[:], rinv[:], start=True, stop=True)
                    rinv_bc = small.tile([128, 512], F32, tag="rinvbc")
                    nc.vector.tensor_copy(out=rinv_bc[:], in_=pbc[:])

                    o_sb = []
                    for ct in range(CH):
                        ot = osb.tile([128, 512], MDT, tag="ot", name=f"ot_{ic}_{ct}" + sfx)
                        if ct < 2:
                            nc.vector.tensor_copy(out=ot[:], in_=po[ct][:])
                        else:
                            nc.scalar.copy(out=ot[:], in_=po[ct][:])
                        o_sb.append(ot)

                    # proj + normalize + residual (x slice read from resident X)
                    for ct in range(CH):
                        py = psc.tile([128, 512], F32, tag="pc", name=f"py_{ic}_{ct}" + sfx)
                        for ch in range(CH):
                            nc.tensor.matmul(
                                py[:], wp_sb[:, ch, ct * 128:(ct + 1) * 128],
                                o_sb[ch][:], start=(ch == 0), stop=(ch == CH - 1),
                            )
                        ft = fin.tile([128, 512], F32, tag="ft", name=f"ft_{ic}_{ct}" + sfx)
                        nc.vector.tensor_mul(out=ft[:], in0=py[:], in1=rinv_bc[:])
                        nc.vector.scalar_tensor_tensor(
                            out=ft[:],
                            in0=X_tiles[ic][:, ct, :].bitcast(F32),
                            scalar=bp_sb[:, ct:ct + 1],
                            in1=ft[:],
                            op0=mybir.AluOpType.add,
                            op1=mybir.AluOpType.add,
                        )
                        nc.sync.dma_start(
                            out=out_r[:, ct, ic * 512:(ic + 1) * 512], in_=ft[:],
                        )

            ctx_psum.close()

    nc.compile()
    return nc


def _prepare_inputs(x, gn_scale, gn_bias, wq, bq, wk, bk, wv, bv, wp, bp):
    x = np.asarray(x, np.float32)
    gn_scale = np.asarray(gn_scale, np.float32)
    gn_bias = np.asarray(gn_bias, np.float32)

    def fold(w, b):
        w = np.asarray(w, np.float32)
        b = np.asarray(b, np.float32)
        return w * gn_scale[None, :], b + w @ gn_bias

    wq2, bq2 = fold(wq, bq)
    wk2, bk2 = fold(wk, bk)
    wv2, bv2 = fold(wv, bv)
    wp2 = np.asarray(wp, np.float32)
    bp2 = np.asarray(bp, np.float32)

    gmat = np.zeros((128, 128), np.float32)
    for g in range(8):
        gmat[g * 16:(g + 1) * 16, g * 16:(g + 1) * 16] = 1.0 / 16.0

    if ATT_BF16:
        import ml_dtypes
        wdt = ml_dtypes.bfloat16
    else:
        wdt = np.float32
    shared = {
        "wqt": np.ascontiguousarray(wq2.T.astype(wdt)),
        "wkt": np.ascontiguousarray(wk2.T.astype(wdt)),
        "wvt": np.ascontiguousarray(wv2.T.astype(wdt)),
        "wpt": np.ascontiguousarray(wp2.T.astype(wdt)),
        "bq": bq2, "bk": bk2, "bv": bv2, "bp": bp2,
        "gmat": gmat,
    }

    xf = x.reshape(B, C, N)
    in_maps = []
    for core in range(8):
        b, qc = divmod(core, 4)
        i0 = qc * I
        xb = xf[b]
        xperm = np.concatenate([xb[:, i0:i0 + I], xb[:, :i0], xb[:, i0 + I:]], axis=1)
        in_maps.append({"x": np.ascontiguousarray(xperm), **shared})
    return in_maps


def _run(in_maps, trace=False):
    if "nc" not in _cached:
        _cached["nc"] = _build()
    return run_bass_kernel_spmd(_cached["nc"], in_maps, list(range(8)), trace=trace)


def kernel(x, gn_scale, gn_bias, wq, bq, wk, bk, wv, bv, wp, bp):
    in_maps = _prepare_inputs(x, gn_scale, gn_bias, wq, bq, wk, bk, wv, bv, wp, bp)
    res = _run(in_maps)
    out = np.empty((B, C, N), np.float32)
    for core in range(8):
        b, qc = divmod(core, 4)
        out[b][:, qc * I:(qc + 1) * I] = res.results[core]["out"]
    return out.reshape(B, C, H, W)



# revision 2
# speedup vs baseline: 2.2550x; 2.2550x over previous
"""AttnBlock (GroupNorm -> single-head 4096x4096 attention -> proj -> residual)
on x:[2,512,64,64] f32, distributed over 8 trn2 NeuronCores.

Sharding: data-parallel over batch (2) x sequence-parallel over query rows
(4 chunks of 1024). Each core receives its batch's full [512, 4096] image with
spatial columns permuted so that its own 1024 query positions are columns
0:1024 (attention and groupnorm are permutation-invariant over spatial
positions, which keeps the SPMD program identical across cores).

All matmul-facing storage is bf16 (fp32 accumulation in PSUM): x streams in
as bf16 (half the head DMA), weights are bf16 (FWL-accelerated loads), and
K/Q/V^T/exp(S) tiles are bf16, which lets V^T stay SBUF-resident (no DRAM
spill/reload). A numpy simulation of this quantization through the reference
gives 5.5e-4 absmax relative error (gate is 2e-2). The residual path reads a
separate fp32 copy of the core's own 1024 columns.

GroupNorm is folded into the q/k/v weights on device (h = (x-mu)*rstd, so
W^T rows are scaled by rstd). Bias handling exploits softmax structure:
  - K bias (and its -W^T mu correction) adds a per-row constant to the
    logits -> cancels in softmax -> skipped entirely.
  - V bias is a per-channel constant on the attention output -> folded into
    the proj bias ON HOST (bp += wp @ bv); only the runtime -W_v^T mu part
    is applied on device, pushed through wp into the final bias with 16
    tiny transposed matmuls.
  - Q bias is applied at Q eviction; its correction (bq - W_q^T mu) is
    computed with 16 tiny transposed matmuls directly in per-partition
    layout (no DRAM-roundtrip transpose).
Softmax normalization: exp tiles accumulate on DVE into bf16 row-partials;
one ones[128,128] matmul broadcasts the cross-partition row sums to all
partitions, and a [128,512] DVE reciprocal feeds the final scale -- all off
the PE critical path.
"""

import numpy as np

import concourse.bass as bass
import concourse.mybir as mybir
import concourse.tile as tile
from concourse import bacc
from concourse.bass_utils import run_bass_kernel_spmd

F32 = mybir.dt.float32
BF16 = mybir.dt.bfloat16

B = 2
C = 512
H = 64
W = 64
N = H * W            # 4096 spatial positions
G = 32               # groups
EPS = 1e-6
CH = 4               # channel chunks of 128
NS = 8               # j slices of 512
JT = 32              # j tiles of 128
I = 1024             # query positions per core
IC = 2               # i chunks of 512 per core
SCALE = float(C) ** -0.5

_cached = {}


def _build(repeat=1):
    nc = bacc.Bacc("TRN2", target_bir_lowering=False, debug=False, num_devices=8)

    x_d = nc.dram_tensor("x", [C, N], BF16, kind="ExternalInput").ap()
    x32_d = nc.dram_tensor("x32", [C, I], F32, kind="ExternalInput").ap()
    wq_d = nc.dram_tensor("wqt", [C, C], BF16, kind="ExternalInput").ap()
    wk_d = nc.dram_tensor("wkt", [C, C], BF16, kind="ExternalInput").ap()
    wv_d = nc.dram_tensor("wvt", [C, C], BF16, kind="ExternalInput").ap()
    wp_d = nc.dram_tensor("wpt", [C, C], BF16, kind="ExternalInput").ap()
    bq_d = nc.dram_tensor("bq", [C], F32, kind="ExternalInput").ap()
    bp_d = nc.dram_tensor("bp", [C], F32, kind="ExternalInput").ap()
    gm_d = nc.dram_tensor("gmat", [128, 128], F32, kind="ExternalInput").ap()
    out_ds = [
        nc.dram_tensor("out" if r == 0 else f"out{r}", [C, I], F32,
                       kind="ExternalOutput").ap()
        for r in range(repeat)
    ]

    x_r = x_d.rearrange("(ch p) n -> p ch n", p=128)       # [128, 4, 4096]
    x32_r = x32_d.rearrange("(ch p) i -> p ch i", p=128)   # [128, 4, 1024]

    with tile.TileContext(nc) as tc:
      for rep in range(repeat):
        sfx = f"_{rep}"
        out_r = out_ds[rep].rearrange("(ch p) i -> p ch i", p=128)
        from contextlib import ExitStack
        ctx_psum = ExitStack()
        with (
            tc.tile_pool(name="consts" + sfx + sfx, bufs=1) as consts,
            tc.tile_pool(name="big" + sfx + sfx, bufs=1) as big,
            tc.tile_pool(name="stats" + sfx + sfx, bufs=1) as stats,
            tc.tile_pool(name="small" + sfx + sfx, bufs=1) as small,
            tc.tile_pool(name="fin" + sfx + sfx, bufs=2) as fin,
        ):
            # ---- persistent constants ----
            bp_sb = consts.tile([128, CH], F32, tag="bp")
            nc.sync.dma_start(out=bp_sb[:], in_=bp_d.rearrange("(ch p) -> p ch", p=128))
            bq_sb = consts.tile([128, CH], F32, tag="bq")
            nc.sync.dma_start(out=bq_sb[:], in_=bq_d.rearrange("(ch p) -> p ch", p=128))
            gm_sb = consts.tile([128, 128], F32, tag="gm")
            nc.sync.dma_start(out=gm_sb[:], in_=gm_d)
            eps_sb = consts.tile([128, 1], F32, tag="eps")
            nc.vector.memset(eps_sb[:], EPS)
            ones128 = consts.tile([128, 128], BF16, tag="ones128")
            nc.vector.memset(ones128[:], 1.0)

            X_tiles = [
                big.tile([128, CH, 512], BF16, tag=f"X{s}", name=f"X{s}" + sfx)
                for s in range(NS)
            ]  # bf16 x, resident, one tile per j-slice for fine-grained deps
            X32_sb = big.tile([128, CH, I], F32, tag="X32")  # fp32 residual slice
            K_sb = big.tile([128, CH, N], BF16, tag="K")     # K[c, j]
            Q_sb = big.tile([128, CH, I], BF16, tag="Q")     # Q[c, i]
            VT_sb = big.tile([128, JT, C], BF16, tag="VT")   # V^T[j, c] resident

            # groupnorm stats tiles
            st_acc = stats.tile([128, CH, NS, 6], F32, tag="stacc")
            mv = stats.tile([128, CH, 2], F32, tag="mv")
            gs = stats.tile([128, CH, 2], F32, tag="gs")
            mean_sb = stats.tile([128, CH], F32, tag="mean")
            rstd_sb = stats.tile([128, CH], F32, tag="rstd")

            psc = ctx_psum.enter_context(
                tc.tile_pool(name="psc" + sfx + sfx, bufs=4, space="PSUM")
            )
            with tc.tile_pool(name="wtmp" + sfx + sfx, bufs=1) as wtmp:
                # ---- phase 1: groupnorm stats over streaming bf16 x ----
                for s in range(NS):
                    dma_eng = nc.sync if s % 2 == 0 else nc.gpsimd
                    dma_eng.dma_start(
                        out=X_tiles[s][:], in_=x_r[:, :, s * 512:(s + 1) * 512],
                    )
                    for ch in range(CH):
                        nc.vector.bn_stats(
                            out=st_acc[:, ch, s, :], in_=X_tiles[s][:, ch, :],
                        )
                # weights are needed only after the stats chain; emitting their
                # DMAs here keeps the x stream at the head of the DMA queues
                wq_sb = [wtmp.tile([128, C], BF16, tag=f"wq{c}", name=f"wq{c}" + sfx) for c in range(CH)]
                wk_sb = [wtmp.tile([128, C], BF16, tag=f"wk{c}", name=f"wk{c}" + sfx) for c in range(CH)]
                wv_sb = [wtmp.tile([128, C], BF16, tag=f"wv{c}", name=f"wv{c}" + sfx) for c in range(CH)]
                for w_sb, w_d in ((wk_sb, wk_d), (wv_sb, wv_d), (wq_sb, wq_d)):
                    w_r = w_d.rearrange("(ch p) o -> p ch o", p=128)
                    for ch in range(CH):
                        nc.sync.dma_start(out=w_sb[ch][:], in_=w_r[:, ch, :])
                # residual fp32 slice + proj weight ride the gpsimd queue; both
                # are consumed only in phase 3
                nc.gpsimd.dma_start(out=X32_sb[:], in_=x32_r[:])
                wp_sb = consts.tile([128, CH, C], BF16, tag="wp")
                nc.gpsimd.dma_start(out=wp_sb[:], in_=wp_d.rearrange("(ch p) o -> p ch o", p=128))

                for ch in range(CH):
                    nc.vector.bn_aggr(out=mv[:, ch, :], in_=st_acc[:, ch, :, :])
                # per-channel (mean, E[x^2]) -> group-averaged via gmat matmul
                nc.vector.tensor_copy(out=gs[:, :, 0], in_=mv[:, :, 0])
                nc.vector.tensor_mul(out=gs[:, :, 1], in0=mv[:, :, 0], in1=mv[:, :, 0])
                nc.vector.tensor_add(out=gs[:, :, 1], in0=gs[:, :, 1], in1=mv[:, :, 1])
                pg = psc.tile([128, CH, 2], F32, tag="pc", bufs=4)
                nc.tensor.matmul(pg[:], gm_sb[:], gs[:], start=True, stop=True)
                nc.vector.tensor_copy(out=mean_sb[:], in_=pg[:, :, 0])
                nc.vector.tensor_mul(out=rstd_sb[:], in0=mean_sb[:], in1=mean_sb[:])
                nc.vector.tensor_sub(out=rstd_sb[:], in0=pg[:, :, 1], in1=rstd_sb[:])
                nc.scalar.activation(
                    out=rstd_sb[:], in_=rstd_sb[:],
                    func=mybir.ActivationFunctionType.Sqrt, bias=eps_sb[:],
                )
                nc.vector.reciprocal(out=rstd_sb[:], in_=rstd_sb[:])

                # ---- fold groupnorm into weights: w' = w * rstd(c_in) ----
                for w_sb in (wk_sb, wv_sb, wq_sb):
                    for ch in range(CH):
                        nc.vector.tensor_scalar_mul(
                            out=w_sb[ch][:], in0=w_sb[ch][:],
                            scalar1=rstd_sb[:, ch:ch + 1],
                        )
                mean_r = stats.tile([128, CH], BF16, tag="meanr")
                nc.vector.tensor_copy(out=mean_r[:], in_=mean_sb[:])
                # dq[o] = sum_c wq'[c,o] mu[c] directly in per-partition layout
                # (o on partitions) via 16 N=1 transposed matmuls; same for dv
                pdq = psc.tile([128, CH], F32, tag="pc", name="pdq" + sfx)
                pdv = psc.tile([128, CH], F32, tag="pc", name="pdv" + sfx)
                for pd, w_sb in ((pdq, wq_sb), (pdv, wv_sb)):
                    for t in range(CH):
                        for ch in range(CH):
                            nc.tensor.matmul(
                                pd[:, t:t + 1],
                                w_sb[ch][:, t * 128:(t + 1) * 128],
                                mean_r[:, ch:ch + 1],
                                start=(ch == 0), stop=(ch == CH - 1),
                            )
                bcorr_q = small.tile([128, CH], F32, tag="bcorrq")
                nc.vector.tensor_sub(out=bcorr_q[:], in0=bq_sb[:], in1=pdq[:])
                dv_sb = small.tile([128, CH], BF16, tag="dv")
                nc.vector.tensor_copy(out=dv_sb[:], in_=pdv[:])

                # ---- phase 2: K / V^T / Q convs from bf16 x + folded weights ----
                for s in range(NS):
                    xsl = X_tiles[s]
                    # K[c_out, j_slice]; no bias (cancels in softmax rows)
                    for t in range(CH):
                        pk = psc.tile([128, 512], F32, tag="pc")
                        for ch in range(CH):
                            nc.tensor.matmul(
                                pk[:], wk_sb[ch][:, t * 128:(t + 1) * 128],
                                xsl[:, ch, :], start=(ch == 0), stop=(ch == CH - 1),
                            )
                        nc.scalar.copy(
                            out=K_sb[:, t, s * 512:(s + 1) * 512], in_=pk[:],
                        )
                    # V^T[j_tile, c] resident in SBUF; bias applied via bp fold
                    for jj in range(4):
                        jt = 4 * s + jj
                        pv = psc.tile([128, 512], F32, tag="pc")
                        for ch in range(CH):
                            nc.tensor.matmul(
                                pv[:], xsl[:, ch, jj * 128:(jj + 1) * 128],
                                wv_sb[ch][:], start=(ch == 0), stop=(ch == CH - 1),
                            )
                        nc.vector.tensor_copy(out=VT_sb[:, jt, :], in_=pv[:])
                    # Q convs ride mid-sweep
                    if s == 3:
                        for sq in range(IC):
                            for t in range(CH):
                                pq = psc.tile([128, 512], F32, tag="pd", name=f"pq_{sq}_{t}" + sfx, bufs=2)
                                for ch in range(CH):
                                    nc.tensor.matmul(
                                        pq[:], wq_sb[ch][:, t * 128:(t + 1) * 128],
                                        X_tiles[sq][:, ch, :], start=(ch == 0), stop=(ch == CH - 1),
                                    )
                                nc.vector.tensor_scalar_add(
                                    out=Q_sb[:, t, sq * 512:(sq + 1) * 512], in0=pq[:],
                                    scalar1=bcorr_q[:, t:t + 1],
                                )

            # effective proj bias: bp (with host-folded V bias) minus the
            # runtime wp^T (wv'^T mu) correction, in per-partition layout
            pcorr = psc.tile([128, CH], F32, tag="pc", name="pcorr" + sfx)
            for t in range(CH):
                for ch in range(CH):
                    nc.tensor.matmul(
                        pcorr[:, t:t + 1],
                        wp_sb[:, ch, t * 128:(t + 1) * 128],
                        dv_sb[:, ch:ch + 1],
                        start=(ch == 0), stop=(ch == CH - 1),
                    )
            bp_eff = small.tile([128, CH], F32, tag="bpeff")
            nc.vector.tensor_sub(out=bp_eff[:], in0=bp_sb[:], in1=pcorr[:])

            # ---- phase 3: attention + proj, per i-chunk of 512 ----
            with (
                tc.tile_pool(name="pexp" + sfx + sfx, bufs=2) as pexp,
                tc.tile_pool(name="osb" + sfx + sfx, bufs=4) as osb,
            ):
                ps_tiles = {}
                emitted = set()
                NPAIR = JT // 2

                def emit_s(ic, pr):
                    # one S-pair: two j-tiles into a double-wide (2-bank) psum
                    emitted.add((ic, pr))
                    qs2 = Q_sb[:, :, ic * 512:(ic + 1) * 512]
                    ps = psc.tile([128, 2, 512], F32, tag="pd", name=f"ps_{ic}_{pr}" + sfx, bufs=2)
                    for u in range(2):
                        jt = 2 * pr + u
                        for ch in range(CH):
                            nc.tensor.matmul(
                                ps[:, u, :], K_sb[:, ch, jt * 128:(jt + 1) * 128],
                                qs2[:, ch, :], start=(ch == 0), stop=(ch == CH - 1),
                            )
                    ps_tiles[(ic, pr)] = ps

                emit_s(0, 0)
                for ic in range(IC):
                    po = [
                        psc.tile([128, 512], F32, tag="pc", name=f"po_{ic}_{ct}" + sfx)
                        for ct in range(CH)
                    ]
                    rs_parts = [
                        small.tile([128, 512], BF16, tag=f"rsacc{k}", name=f"rs_{ic}_{k}" + sfx)
                        for k in range(2)
                    ]

                    # software-pipelined: emit S(pr+1) before O(pr) so the PE
                    # never waits on the ACT exp of the current tile; at the
                    # end of a chunk, prefetch the next chunk's first S tiles
                    # so the PE has work during the DVE-heavy epilogue
                    for pr in range(NPAIR):
                        # one exp instruction covers both j-tiles of the pair
                        pt = pexp.tile([128, 2, 512], BF16, tag="pt", name=f"pt_{ic}_{pr}" + sfx)
                        nc.scalar.activation(
                            out=pt[:], in_=ps_tiles.pop((ic, pr))[:],
                            func=mybir.ActivationFunctionType.Exp, scale=SCALE,
                        )
                        if pr + 1 < NPAIR:
                            if (ic, pr + 1) not in emitted:
                                emit_s(ic, pr + 1)
                        elif ic + 1 < IC:
                            emit_s(ic + 1, 0)
                        for u in range(2):
                            jt = 2 * pr + u
                            vt = VT_sb[:, jt, :]
                            for ct in range(CH):
                                nc.tensor.matmul(
                                    po[ct][:], vt[:, ct * 128:(ct + 1) * 128], pt[:, u, :],
                                    start=(jt == 0), stop=(jt == JT - 1),
                                )
                            rs_k = rs_parts[u]
                            if pr == 0:
                                nc.vector.tensor_copy(out=rs_k[:], in_=pt[:, u, :])
                            else:
                                nc.vector.tensor_add(out=rs_k[:], in0=rs_k[:], in1=pt[:, u, :])

                    # cross-partition row sums broadcast to all partitions with
                    # one ones[128,128] matmul, then one [128,512] reciprocal
                    nc.vector.tensor_add(
                        out=rs_parts[0][:], in0=rs_parts[0][:], in1=rs_parts[1][:]
                    )
                    pbs = psc.tile([128, 512], F32, tag="pc", name=f"pbs_{ic}" + sfx)
                    nc.tensor.matmul(pbs[:], ones128[:], rs_parts[0][:], start=True, stop=True)
                    rinv_bc = small.tile([128, 512], F32, tag="rinvbc")
                    nc.vector.reciprocal(out=rinv_bc[:], in_=pbs[:])

                    # O evictions all on the scalar engine so the (slow) DVE
                    # reciprocal cannot block the proj matmuls
                    o_sb = []
                    for ct in range(CH):
                        ot = osb.tile([128, 512], BF16, tag="ot", name=f"ot_{ic}_{ct}" + sfx)
                        nc.scalar.copy(out=ot[:], in_=po[ct][:])
                        o_sb.append(ot)

                    # proj + normalize + residual (fp32 x slice)
                    for ct in range(CH):
                        py = psc.tile([128, 512], F32, tag="pd", name=f"py_{ic}_{ct}" + sfx, bufs=2)
                        for ch in range(CH):
                            nc.tensor.matmul(
                                py[:], wp_sb[:, ch, ct * 128:(ct + 1) * 128],
                                o_sb[ch][:], start=(ch == 0), stop=(ch == CH - 1),
                            )
                        ft = fin.tile([128, 512], F32, tag="ft", name=f"ft_{ic}_{ct}" + sfx)
                        nc.vector.tensor_mul(out=ft[:], in0=py[:], in1=rinv_bc[:])
                        nc.vector.scalar_tensor_tensor(
                            out=ft[:],
                            in0=X32_sb[:, ct, ic * 512:(ic + 1) * 512],
                            scalar=bp_eff[:, ct:ct + 1],
                            in1=ft[:],
                            op0=mybir.AluOpType.add,
                            op1=mybir.AluOpType.add,
                        )
                        nc.sync.dma_start(
                            out=out_r[:, ct, ic * 512:(ic + 1) * 512], in_=ft[:],
                        )

            ctx_psum.close()

    nc.compile()
    return nc


def _prepare_inputs(x, gn_scale, gn_bias, wq, bq, wk, bk, wv, bv, wp, bp):
    import ml_dtypes
    x = np.asarray(x, np.float32)
    gn_scale = np.asarray(gn_scale, np.float32)
    gn_bias = np.asarray(gn_bias, np.float32)

    def fold(w, b):
        w = np.asarray(w, np.float32)
        b = np.asarray(b, np.float32)
        return w * gn_scale[None, :], b + w @ gn_bias

    wq2, bq2 = fold(wq, bq)
    wk2, _ = fold(wk, bk)          # K bias cancels in softmax -> dropped
    wv2, bv2 = fold(wv, bv)
    wp2 = np.asarray(wp, np.float32)
    # V bias is a per-channel constant on the attention output: push it
    # through the projection into bp on the host
    bp2 = np.asarray(bp, np.float32) + wp2 @ bv2

    gmat = np.zeros((128, 128), np.float32)
    for g in range(8):
        gmat[g * 16:(g + 1) * 16, g * 16:(g + 1) * 16] = 1.0 / 16.0

    bf = ml_dtypes.bfloat16
    shared = {
        "wqt": np.ascontiguousarray(wq2.T.astype(bf)),
        "wkt": np.ascontiguousarray(wk2.T.astype(bf)),
        "wvt": np.ascontiguousarray(wv2.T.astype(bf)),
        "wpt": np.ascontiguousarray(wp2.T.astype(bf)),
        "bq": bq2, "bp": bp2,
        "gmat": gmat,
    }

    xf = x.reshape(B, C, N)
    in_maps = []
    for core in range(8):
        b, qc = divmod(core, 4)
        i0 = qc * I
        xb = xf[b]
        xperm = np.concatenate([xb[:, i0:i0 + I], xb[:, :i0], xb[:, i0 + I:]], axis=1)
        in_maps.append({
            "x": np.ascontiguousarray(xperm.astype(bf)),
            "x32": np.ascontiguousarray(xb[:, i0:i0 + I]),
            **shared,
        })
    return in_maps


def _run(in_maps, trace=False):
    if "nc" not in _cached:
        _cached["nc"] = _build()
    return run_bass_kernel_spmd(_cached["nc"], in_maps, list(range(8)), trace=trace)


def kernel(x, gn_scale, gn_bias, wq, bq, wk, bk, wv, bv, wp, bp):
    in_maps = _prepare_inputs(x, gn_scale, gn_bias, wq, bq, wk, bk, wv, bv, wp, bp)
    res = _run(in_maps)
    out = np.empty((B, C, N), np.float32)
    for core in range(8):
        b, qc = divmod(core, 4)
        out[b][:, qc * I:(qc + 1) * I] = res.results[core]["out"]
    return out.reshape(B, C, H, W)


# revision 8
# speedup vs baseline: 2.9852x; 1.3238x over previous
"""AttnBlock (GroupNorm -> single-head 4096x4096 attention -> proj -> residual)
on x:[2,512,64,64] f32, distributed over 8 trn2 NeuronCores.

Sharding: data-parallel over batch (2) x sequence-parallel over query rows
(4 chunks of 1024). Each core receives its batch's full [512, 4096] image with
spatial columns permuted so that its own 1024 query positions are columns
0:1024 (attention and groupnorm are permutation-invariant over spatial
positions, which keeps the SPMD program identical across cores).

All matmul-facing storage is bf16 (fp32 accumulation in PSUM): x streams in
as bf16 (half the head DMA), weights are bf16 (FWL-accelerated loads), and
K/Q/V^T/exp(S) tiles are bf16, which lets V^T stay SBUF-resident (no DRAM
spill/reload). A numpy simulation of this quantization through the reference
gives 5.5e-4 absmax relative error (gate is 2e-2). The residual path reads a
separate fp32 copy of the core's own 1024 columns.

GroupNorm is folded into the q/k/v weights on device (h = (x-mu)*rstd, so
W^T rows are scaled by rstd). Bias handling exploits softmax structure:
  - K bias (and its -W^T mu correction) adds a per-row constant to the
    logits -> cancels in softmax -> skipped entirely.
  - V bias is a per-channel constant on the attention output -> folded into
    the proj bias ON HOST (bp += wp @ bv); only the runtime -W_v^T mu part
    is applied on device, pushed through wp into the final bias with 16
    tiny transposed matmuls.
  - Q bias is applied at Q eviction; its correction (bq - W_q^T mu) is
    computed with 16 tiny transposed matmuls directly in per-partition
    layout (no DRAM-roundtrip transpose).
Softmax normalization: exp tiles accumulate on DVE into bf16 row-partials;
one ones[128,128] matmul broadcasts the cross-partition row sums to all
partitions, and a [128,512] DVE reciprocal feeds the final scale -- all off
the PE critical path.
"""

import numpy as np

import concourse.bass as bass
import concourse.mybir as mybir
import concourse.tile as tile
from concourse import bacc
from concourse.bass_utils import run_bass_kernel_spmd

F32 = mybir.dt.float32
BF16 = mybir.dt.bfloat16

B = 2
C = 512
H = 64
W = 64
N = H * W            # 4096 spatial positions
G = 32               # groups
EPS = 1e-6
CH = 4               # channel chunks of 128
NS = 8               # j slices of 512
JT = 32              # j tiles of 128
I = 1024             # query positions per core
IC = 2               # i chunks of 512 per core
SCALE = float(C) ** -0.5

_cached = {}


def _build(repeat=1):
    nc = bacc.Bacc("TRN2", target_bir_lowering=False, debug=False, num_devices=8)

    # all inputs are host-relayouted to partition-major so every DMA is a
    # straight [128, k] copy with fully contiguous per-partition lines
    x_d = nc.dram_tensor("x", [128, CH, N], BF16, kind="ExternalInput").ap()
    x32_d = nc.dram_tensor("x32", [128, CH, I], F32, kind="ExternalInput").ap()
    wq_d = nc.dram_tensor("wqt", [128, CH, C], BF16, kind="ExternalInput").ap()
    wk_d = nc.dram_tensor("wkt", [128, CH, C], BF16, kind="ExternalInput").ap()
    wv_d = nc.dram_tensor("wvt", [128, CH, C], BF16, kind="ExternalInput").ap()
    wp_d = nc.dram_tensor("wpt", [128, CH, C], BF16, kind="ExternalInput").ap()
    bq_d = nc.dram_tensor("bq", [128, CH], F32, kind="ExternalInput").ap()
    bp_d = nc.dram_tensor("bp", [128, CH], F32, kind="ExternalInput").ap()
    gm_d = nc.dram_tensor("gmat", [128, 128], F32, kind="ExternalInput").ap()
    out_ds = [
        nc.dram_tensor("out" if r == 0 else f"out{r}", [C, I], F32,
                       kind="ExternalOutput").ap()
        for r in range(repeat)
    ]

    x_r = x_d       # [128, 4, 4096]
    x32_r = x32_d   # [128, 4, 1024]

    with tile.TileContext(nc) as tc:
      for rep in range(repeat):
        sfx = f"_{rep}"
        out_r = out_ds[rep].rearrange("(ch p) i -> p ch i", p=128)
        from contextlib import ExitStack
        ctx_psum = ExitStack()
        with (
            tc.tile_pool(name="consts" + sfx + sfx, bufs=1) as consts,
            tc.tile_pool(name="big" + sfx + sfx, bufs=1) as big,
            tc.tile_pool(name="stats" + sfx + sfx, bufs=1) as stats,
            tc.tile_pool(name="small" + sfx + sfx, bufs=1) as small,
            tc.tile_pool(name="fin" + sfx + sfx, bufs=2) as fin,
        ):
            # ---- persistent constants (on the vector DMA queue, off the
            # x-critical sync/gpsimd queues) ----
            bp_sb = consts.tile([128, CH], F32, tag="bp")
            nc.scalar.dma_start(out=bp_sb[:], in_=bp_d)
            bq_sb = consts.tile([128, CH], F32, tag="bq")
            nc.scalar.dma_start(out=bq_sb[:], in_=bq_d)
            gm_sb = consts.tile([128, 128], F32, tag="gm")
            nc.scalar.dma_start(out=gm_sb[:], in_=gm_d)
            eps_sb = consts.tile([128, 1], F32, tag="eps")
            nc.vector.memset(eps_sb[:], EPS)
            ones128 = consts.tile([128, 128], BF16, tag="ones128")
            nc.vector.memset(ones128[:], 1.0)

            X_tiles = [
                big.tile([128, CH, 512], BF16, tag=f"X{s}", name=f"X{s}" + sfx)
                for s in range(NS)
            ]  # bf16 x, resident, one tile per j-slice for fine-grained deps
            X32_sb = big.tile([128, CH, I], F32, tag="X32")  # fp32 residual slice
            K_sb = big.tile([128, CH, N], BF16, tag="K")     # K[c, j]
            Q_sb = big.tile([128, CH, I], BF16, tag="Q")     # Q[c, i]
            VT_sb = big.tile([128, JT, C], BF16, tag="VT")   # V^T[j, c] resident

            # groupnorm stats tiles; stats are split: slices 0..4 via DVE
            # bn_stats, slices 5..7 via ACT Square/Copy accumulations
            NSV = 5                       # slices handled by DVE bn_stats
            NSA = NS - NSV                # slices handled by ACT accum_out
            st_acc = stats.tile([128, CH, NSV, 6], F32, tag="stacc")
            sma = stats.tile([128, CH, NSA], F32, tag="sma")
            sqa = stats.tile([128, CH, NSA], F32, tag="sqa")
            sm2 = stats.tile([128, CH, 1], F32, tag="sm2")
            sq2 = stats.tile([128, CH, 1], F32, tag="sq2")
            scr = stats.tile([128, 512], BF16, tag="scr")
            mv = stats.tile([128, CH, 2], F32, tag="mv")
            q1 = stats.tile([128, CH], F32, tag="q1")
            gs = stats.tile([128, CH, 2], F32, tag="gs")
            mean_sb = stats.tile([128, CH], F32, tag="mean")
            rstd_sb = stats.tile([128, CH], F32, tag="rstd")

            psc = ctx_psum.enter_context(
                tc.tile_pool(name="psc" + sfx + sfx, bufs=4, space="PSUM")
            )
            with tc.tile_pool(name="wtmp" + sfx + sfx, bufs=1) as wtmp:
                # ---- phase 1: groupnorm stats over streaming bf16 x ----
                for s in range(NS):
                    dma_eng = nc.sync if s % 2 == 0 else nc.gpsimd
                    dma_eng.dma_start(
                        out=X_tiles[s][:], in_=x_r[:, :, s * 512:(s + 1) * 512],
                    )
                    for ch in range(CH):
                        if s < NSV:
                            nc.vector.bn_stats(
                                out=st_acc[:, ch, s, :], in_=X_tiles[s][:, ch, :],
                            )
                        else:
                            a = s - NSV
                            nc.scalar.activation(
                                out=scr[:], in_=X_tiles[s][:, ch, :],
                                func=mybir.ActivationFunctionType.Square,
                                accum_out=sqa[:, ch, a:a + 1],
                            )
                            nc.scalar.activation(
                                out=scr[:], in_=X_tiles[s][:, ch, :],
                                func=mybir.ActivationFunctionType.Copy,
                                accum_out=sma[:, ch, a:a + 1],
                            )
                # weights are needed only after the stats chain; emitting their
                # DMAs here keeps the x stream at the head of the DMA queues
                wk_sb4 = wtmp.tile([128, CH, C], BF16, tag="wk4")
                wv_sb4 = wtmp.tile([128, CH, C], BF16, tag="wv4")
                wq_sb4 = wtmp.tile([128, CH, C], BF16, tag="wq4")
                nc.sync.dma_start(out=wk_sb4[:], in_=wk_d)
                nc.sync.dma_start(out=wv_sb4[:], in_=wv_d)
                nc.sync.dma_start(out=wq_sb4[:], in_=wq_d)
                wk_sb = [wk_sb4[:, c, :] for c in range(CH)]
                wv_sb = [wv_sb4[:, c, :] for c in range(CH)]
                wq_sb = [wq_sb4[:, c, :] for c in range(CH)]
                # residual fp32 slice + proj weight ride the vector DMA queue;
                # both are consumed only in phase 3
                nc.scalar.dma_start(out=X32_sb[:], in_=x32_r[:])
                wp_sb = consts.tile([128, CH, C], BF16, tag="wp")
                nc.scalar.dma_start(out=wp_sb[:], in_=wp_d)

                for ch in range(CH):
                    nc.vector.bn_aggr(out=mv[:, ch, :], in_=st_acc[:, ch, :, :])
                nc.vector.tensor_reduce(
                    out=sm2[:], in_=sma[:], axis=mybir.AxisListType.X,
                    op=mybir.AluOpType.add,
                )
                nc.vector.tensor_reduce(
                    out=sq2[:], in_=sqa[:], axis=mybir.AxisListType.X,
                    op=mybir.AluOpType.add,
                )
                # combine the two partial stats into per-channel E[x], E[x^2]
                # (DVE part covers NSV*512 elems, ACT part the rest), then
                # group-average via the gmat matmul
                W1 = float(NSV * 512) / float(N)
                W2 = 1.0 / float(N)
                nc.scalar.mul(out=gs[:, :, 0], in_=mv[:, :, 0], mul=W1)
                nc.vector.scalar_tensor_tensor(
                    out=gs[:, :, 0], in0=sm2[:, :, 0], scalar=W2, in1=gs[:, :, 0],
                    op0=mybir.AluOpType.mult, op1=mybir.AluOpType.add,
                )
                nc.vector.tensor_mul(out=q1[:], in0=mv[:, :, 0], in1=mv[:, :, 0])
                nc.vector.tensor_add(out=q1[:], in0=q1[:], in1=mv[:, :, 1])
                nc.scalar.mul(out=gs[:, :, 1], in_=q1[:], mul=W1)
                nc.vector.scalar_tensor_tensor(
                    out=gs[:, :, 1], in0=sq2[:, :, 0], scalar=W2, in1=gs[:, :, 1],
                    op0=mybir.AluOpType.mult, op1=mybir.AluOpType.add,
                )
                pg = psc.tile([128, CH, 2], F32, tag="pc", bufs=4)
                nc.tensor.matmul(pg[:], gm_sb[:], gs[:], start=True, stop=True)
                nc.vector.tensor_copy(out=mean_sb[:], in_=pg[:, :, 0])
                nc.vector.tensor_mul(out=rstd_sb[:], in0=mean_sb[:], in1=mean_sb[:])
                nc.vector.tensor_sub(out=rstd_sb[:], in0=pg[:, :, 1], in1=rstd_sb[:])
                nc.scalar.activation(
                    out=rstd_sb[:], in_=rstd_sb[:],
                    func=mybir.ActivationFunctionType.Sqrt, bias=eps_sb[:],
                )
                nc.vector.reciprocal(out=rstd_sb[:], in_=rstd_sb[:])

                # ---- fold groupnorm into weights: w' = w * rstd(c_in) ----
                for w_sb in (wk_sb, wv_sb, wq_sb):
                    for ch in range(CH):
                        nc.vector.tensor_scalar_mul(
                            out=w_sb[ch][:], in0=w_sb[ch][:],
                            scalar1=rstd_sb[:, ch:ch + 1],
                        )
                mean_r = stats.tile([128, CH], BF16, tag="meanr")
                nc.vector.tensor_copy(out=mean_r[:], in_=mean_sb[:])
                # dq[o] = sum_c wq'[c,o] mu[c] directly in per-partition layout
                # (o on partitions) via 16 N=1 transposed matmuls; same for dv
                pdq = psc.tile([128, CH], F32, tag="pc", name="pdq" + sfx)
                pdv = psc.tile([128, CH], F32, tag="pc", name="pdv" + sfx)
                for pd, w_sb in ((pdq, wq_sb), (pdv, wv_sb)):
                    for t in range(CH):
                        for ch in range(CH):
                            nc.tensor.matmul(
                                pd[:, t:t + 1],
                                w_sb[ch][:, t * 128:(t + 1) * 128],
                                mean_r[:, ch:ch + 1],
                                start=(ch == 0), stop=(ch == CH - 1),
                            )
                bcorr_q = small.tile([128, CH], F32, tag="bcorrq")
                nc.vector.tensor_sub(out=bcorr_q[:], in0=bq_sb[:], in1=pdq[:])
                dv_sb = small.tile([128, CH], BF16, tag="dv")
                nc.vector.tensor_copy(out=dv_sb[:], in_=pdv[:])

                # ---- phase 2: K / V^T / Q convs from bf16 x + folded weights ----
                for s in range(NS):
                    xsl = X_tiles[s]
                    # K[c_out, j_slice]; no bias (cancels in softmax rows)
                    for t in range(CH):
                        pk = psc.tile([128, 512], F32, tag="pc")
                        for ch in range(CH):
                            nc.tensor.matmul(
                                pk[:], wk_sb[ch][:, t * 128:(t + 1) * 128],
                                xsl[:, ch, :], start=(ch == 0), stop=(ch == CH - 1),
                            )
                        nc.scalar.copy(
                            out=K_sb[:, t, s * 512:(s + 1) * 512], in_=pk[:],
                        )
                    # V^T[j_tile, c] resident in SBUF; bias applied via bp fold
                    for jj in range(4):
                        jt = 4 * s + jj
                        pv = psc.tile([128, 512], F32, tag="pc")
                        for ch in range(CH):
                            nc.tensor.matmul(
                                pv[:], xsl[:, ch, jj * 128:(jj + 1) * 128],
                                wv_sb[ch][:], start=(ch == 0), stop=(ch == CH - 1),
                            )
                        nc.vector.tensor_copy(out=VT_sb[:, jt, :], in_=pv[:])
                    # Q convs ride mid-sweep
                    if s == 3:
                        for sq in range(IC):
                            for t in range(CH):
                                pq = psc.tile([128, 512], F32, tag="pd", name=f"pq_{sq}_{t}" + sfx, bufs=2)
                                for ch in range(CH):
                                    nc.tensor.matmul(
                                        pq[:], wq_sb[ch][:, t * 128:(t + 1) * 128],
                                        X_tiles[sq][:, ch, :], start=(ch == 0), stop=(ch == CH - 1),
                                    )
                                nc.vector.tensor_scalar_add(
                                    out=Q_sb[:, t, sq * 512:(sq + 1) * 512], in0=pq[:],
                                    scalar1=bcorr_q[:, t:t + 1],
                                )

            # effective proj bias: bp (with host-folded V bias) minus the
            # runtime wp^T (wv'^T mu) correction, in per-partition layout
            pcorr = psc.tile([128, CH], F32, tag="pc", name="pcorr" + sfx)
            for t in range(CH):
                for ch in range(CH):
                    nc.tensor.matmul(
                        pcorr[:, t:t + 1],
                        wp_sb[:, ch, t * 128:(t + 1) * 128],
                        dv_sb[:, ch:ch + 1],
                        start=(ch == 0), stop=(ch == CH - 1),
                    )
            bp_eff = small.tile([128, CH], F32, tag="bpeff")
            nc.vector.tensor_sub(out=bp_eff[:], in0=bp_sb[:], in1=pcorr[:])

            # ---- phase 3: attention + proj, per i-chunk of 512 ----
            with (
                tc.tile_pool(name="pexp" + sfx + sfx, bufs=2) as pexp,
                tc.tile_pool(name="osb" + sfx + sfx, bufs=4) as osb,
            ):
                ps_tiles = {}
                emitted = set()
                NPAIR = JT // 2

                def emit_s(ic, pr):
                    # one S-pair: two j-tiles into a double-wide (2-bank) psum
                    emitted.add((ic, pr))
                    qs2 = Q_sb[:, :, ic * 512:(ic + 1) * 512]
                    ps = psc.tile([128, 2, 512], F32, tag="pd", name=f"ps_{ic}_{pr}" + sfx, bufs=2)
                    for u in range(2):
                        jt = 2 * pr + u
                        for ch in range(CH):
                            nc.tensor.matmul(
                                ps[:, u, :], K_sb[:, ch, jt * 128:(jt + 1) * 128],
                                qs2[:, ch, :], start=(ch == 0), stop=(ch == CH - 1),
                            )
                    ps_tiles[(ic, pr)] = ps

                emit_s(0, 0)
                for ic in range(IC):
                    po = [
                        psc.tile([128, 512], F32, tag="pc", name=f"po_{ic}_{ct}" + sfx)
                        for ct in range(CH)
                    ]
                    rs_parts = [
                        small.tile([128, 512], BF16, tag=f"rsacc{k}", name=f"rs_{ic}_{k}" + sfx)
                        for k in range(2)
                    ]

                    # software-pipelined: emit S(pr+1) before O(pr) so the PE
                    # never waits on the ACT exp of the current tile; at the
                    # end of a chunk, prefetch the next chunk's first S tiles
                    # so the PE has work during the DVE-heavy epilogue
                    for pr in range(NPAIR):
                        # one exp instruction covers both j-tiles of the pair
                        pt = pexp.tile([128, 2, 512], BF16, tag="pt", name=f"pt_{ic}_{pr}" + sfx)
                        nc.scalar.activation(
                            out=pt[:], in_=ps_tiles.pop((ic, pr))[:],
                            func=mybir.ActivationFunctionType.Exp, scale=SCALE,
                        )
                        if pr + 1 < NPAIR:
                            if (ic, pr + 1) not in emitted:
                                emit_s(ic, pr + 1)
                        elif ic + 1 < IC:
                            emit_s(ic + 1, 0)
                        for u in range(2):
                            jt = 2 * pr + u
                            vt = VT_sb[:, jt, :]
                            for ct in range(CH):
                                nc.tensor.matmul(
                                    po[ct][:], vt[:, ct * 128:(ct + 1) * 128], pt[:, u, :],
                                    start=(jt == 0), stop=(jt == JT - 1),
                                )
                            rs_k = rs_parts[u]
                            if pr == 0:
                                nc.vector.tensor_copy(out=rs_k[:], in_=pt[:, u, :])
                            else:
                                nc.vector.tensor_add(out=rs_k[:], in0=rs_k[:], in1=pt[:, u, :])

                    # cross-partition row sums broadcast to all partitions with
                    # one ones[128,128] matmul, then one [128,512] reciprocal
                    nc.vector.tensor_add(
                        out=rs_parts[0][:], in0=rs_parts[0][:], in1=rs_parts[1][:]
                    )
                    pbs = psc.tile([128, 512], F32, tag="pc", name=f"pbs_{ic}" + sfx)
                    nc.tensor.matmul(pbs[:], ones128[:], rs_parts[0][:], start=True, stop=True)
                    rinv_bc = small.tile([128, 512], F32, tag="rinvbc")
                    nc.vector.reciprocal(out=rinv_bc[:], in_=pbs[:])

                    # O evictions all on the scalar engine so the (slow) DVE
                    # reciprocal cannot block the proj matmuls
                    o_sb = []
                    for ct in range(CH):
                        ot = osb.tile([128, 512], BF16, tag="ot", name=f"ot_{ic}_{ct}" + sfx)
                        nc.scalar.copy(out=ot[:], in_=po[ct][:])
                        o_sb.append(ot)

                    # proj + normalize + residual (fp32 x slice)
                    for ct in range(CH):
                        py = psc.tile([128, 512], F32, tag="pd", name=f"py_{ic}_{ct}" + sfx, bufs=2)
                        for ch in range(CH):
                            nc.tensor.matmul(
                                py[:], wp_sb[:, ch, ct * 128:(ct + 1) * 128],
                                o_sb[ch][:], start=(ch == 0), stop=(ch == CH - 1),
                            )
                        ft = fin.tile([128, 512], F32, tag="ft", name=f"ft_{ic}_{ct}" + sfx)
                        nc.vector.tensor_mul(out=ft[:], in0=py[:], in1=rinv_bc[:])
                        nc.vector.scalar_tensor_tensor(
                            out=ft[:],
                            in0=X32_sb[:, ct, ic * 512:(ic + 1) * 512],
                            scalar=bp_eff[:, ct:ct + 1],
                            in1=ft[:],
                            op0=mybir.AluOpType.add,
                            op1=mybir.AluOpType.add,
                        )
                        nc.sync.dma_start(
                            out=out_r[:, ct, ic * 512:(ic + 1) * 512], in_=ft[:],
                        )

            ctx_psum.close()

    nc.compile()
    return nc


def _prepare_inputs(x, gn_scale, gn_bias, wq, bq, wk, bk, wv, bv, wp, bp):
    import ml_dtypes
    x = np.asarray(x, np.float32)
    gn_scale = np.asarray(gn_scale, np.float32)
    gn_bias = np.asarray(gn_bias, np.float32)

    def fold(w, b):
        w = np.asarray(w, np.float32)
        b = np.asarray(b, np.float32)
        return w * gn_scale[None, :], b + w @ gn_bias

    wq2, bq2 = fold(wq, bq)
    wk2, _ = fold(wk, bk)          # K bias cancels in softmax -> dropped
    wv2, bv2 = fold(wv, bv)
    wp2 = np.asarray(wp, np.float32)
    # V bias is a per-channel constant on the attention output: push it
    # through the projection into bp on the host
    bp2 = np.asarray(bp, np.float32) + wp2 @ bv2

    gmat = np.zeros((128, 128), np.float32)
    for g in range(8):
        gmat[g * 16:(g + 1) * 16, g * 16:(g + 1) * 16] = 1.0 / 16.0

    bf = ml_dtypes.bfloat16

    def pmaj(a):
        # [C, k] -> [128, CH, k] partition-major (channel c = ch*128 + p)
        return np.ascontiguousarray(a.reshape(CH, 128, -1).transpose(1, 0, 2))

    shared = {
        "wqt": pmaj(wq2.T.astype(bf)),
        "wkt": pmaj(wk2.T.astype(bf)),
        "wvt": pmaj(wv2.T.astype(bf)),
        "wpt": pmaj(wp2.T.astype(bf)),
        "bq": np.ascontiguousarray(bq2.reshape(CH, 128).T),
        "bp": np.ascontiguousarray(bp2.reshape(CH, 128).T),
        "gmat": gmat,
    }

    xf = x.reshape(B, C, N)
    in_maps = []
    for core in range(8):
        b, qc = divmod(core, 4)
        i0 = qc * I
        xb = xf[b]
        xperm = np.concatenate([xb[:, i0:i0 + I], xb[:, :i0], xb[:, i0 + I:]], axis=1)
        in_maps.append({
            "x": pmaj(xperm.astype(bf)),
            "x32": pmaj(xb[:, i0:i0 + I]),
            **shared,
        })
    return in_maps


def _run(in_maps, trace=False):
    if "nc" not in _cached:
        _cached["nc"] = _build()
    return run_bass_kernel_spmd(_cached["nc"], in_maps, list(range(8)), trace=trace)


def kernel(x, gn_scale, gn_bias, wq, bq, wk, bk, wv, bv, wp, bp):
    in_maps = _prepare_inputs(x, gn_scale, gn_bias, wq, bq, wk, bk, wv, bv, wp, bp)
    res = _run(in_maps)
    out = np.empty((B, C, N), np.float32)
    for core in range(8):
        b, qc = divmod(core, 4)
        out[b][:, qc * I:(qc + 1) * I] = res.results[core]["out"]
    return out.reshape(B, C, H, W)
